# revision 1
# baseline (speedup 1.0000x reference)
"""Block-sparse attention (local + vertical-strided causal mask) on 8 TRN2 cores.

Sharding: one head per NeuronCore (H=8, n_cores=8).

Per-core device algorithm (head h, residue r = 7-h):
  The 4096x4096 score matrix is processed at 128x128 granularity:
  "pair" i = q block-rows (2i, 2i+1) (128 q tokens), "chunk" = 128 k tokens
  (2 mask blocks of 64). Local window -> chunks c in [i-8, i] of K itself;
  vertical-strided blocks -> host-gathered K_vert (6 blocks of 64, kb = 8j+r),
  processed as 3 chunks shared by all cores, with per-core validity applied
  as multiplicative 0/1 per-partition scalars.

  S^T orientation: S^T[k,q] = kT_chunk.T @ qT_pair  (PE, bf16)
  P^T = exp(sm_scale * S^T)                          (ACT, one call per PSUM group)
  masks (triangle / window-start / vert validity)    (DVE)
  out[q,0:128] += P^T_chunk.T @ [V | 1]_chunk        (PE, PSUM-accumulated)
  col 128 of out = softmax denominator; normalize with per-partition
  reciprocal + tensor_scalar multiply, DMA out [q, d] per pair.
"""

import numpy as np
import ml_dtypes

BF16 = ml_dtypes.bfloat16

H = 8
S = 4096
D = 128
BLK = 64
NB = S // BLK        # 64 block rows
NPAIR = NB // 2      # 32 row pairs
NVSLOT = 6           # usable vertical slots (kb = 8j + r <= 47)
NVC = NVSLOT // 2    # 3 vertical chunks
GROUP = 8            # PSUM staging slots per exp group (8 * 128 f32 = 2 banks)

NEG = -30000.0


def make_schedule():
    """Global ordered visit list. visit = (kind, idx, pair)
    kind "local": idx = chunk c (k blocks 2c, 2c+1), pairs i in [c, c+8]
    kind "vert":  idx = vc (K_vert slots 2vc, 2vc+1)
    Vert visits for pairs [c0, c0+8) are inserted right before local chunk
    c0 in {8, 16, 24}, after all their opening local chunks."""
    visits = []
    for c in range(NPAIR):
        if c in (8, 16, 24):
            for vc in range(NVC):
                if 8 * vc + 8 <= c:
                    for i in range(c, c + 8):
                        visits.append(("vert", vc, i))
        for i in range(c, min(c + 8, NPAIR - 1) + 1):
            visits.append(("local", c, i))
    return visits


def vert_visit_order():
    return [(vc_, i_) for (kind, vc_, i_) in make_schedule() if kind == "vert"]


_PROGRAM = None


def _build_program(loop_n=None, ablate=(), pv_delay=3, group=GROUP, stage_bufs=2,
                   pt_bufs=None, exp_split=1, dma_split=16, qt_gpsimd=True,
                   ob_bufs=3, rd_bufs=4, vaug_gpsimd=False, store_sync=False):
    if pt_bufs is None:
        pt_bufs = pv_delay + 2
    """Build the SPMD program. loop_n: wrap the whole body (incl. input DMA)
    in an in-NEFF For loop with that trip count — used only for timing.
    ablate: subset of {"masks","pv","epi","exp"} — drop stages (timing only).
    pv_delay: groups of software-pipeline delay between S^T and PV.
    exp_split: number of ACT calls per group."""
    import contextlib
    import concourse.bass as bass
    import concourse.mybir as mybir
    import concourse.tile as tile
    from concourse import bacc

    fp32 = mybir.dt.float32
    bf16 = mybir.dt.bfloat16

    nc = bacc.Bacc("TRN2", target_bir_lowering=False, debug=False, num_devices=H)

    qt_d = nc.dram_tensor("qt", [D, S], bf16, kind="ExternalInput").ap()
    kt_d = nc.dram_tensor("kt", [D, S], bf16, kind="ExternalInput").ap()
    ktv_d = nc.dram_tensor("ktv", [D, NVSLOT * BLK], bf16, kind="ExternalInput").ap()
    vaug_d = nc.dram_tensor("vaug", [128, NPAIR, D + 1], bf16, kind="ExternalInput").ap()
    vvaug_d = nc.dram_tensor("vvaug", [128, NVC, D + 1], bf16, kind="ExternalInput").ap()
    vs_d = nc.dram_tensor("vs", [128, 48, 2], fp32, kind="ExternalInput").ap()
    tri_d = nc.dram_tensor("tri", [128, 128], bf16, kind="ExternalInput").ap()
    mstart_d = nc.dram_tensor("mstart", [128, 128], bf16, kind="ExternalInput").ap()
    smsc_d = nc.dram_tensor("smsc", [128, 1], fp32, kind="ExternalInput").ap()
    o_d = nc.dram_tensor("o", [S, D], fp32, kind="ExternalOutput").ap()

    visits = make_schedule()
    # first/last visit index per pair
    first = {}
    last = {}
    for g, (kind, idx, i) in enumerate(visits):
        first.setdefault(i, g)
        last[i] = g
    # PSUM start_tensor_calc zeroes the full 2KB bank (zero-region), so only
    # the first matmul touching an oacc tile may carry start=True.
    tile_first = {}
    for g, (kind, idx, i) in enumerate(visits):
        tile_first.setdefault(i // 3, g)
    with tile.TileContext(nc) as tc:
        with (
            tc.tile_pool(name="big", bufs=1) as big,
            tc.tile_pool(name="stage", bufs=stage_bufs, space="PSUM") as stagep,
            tc.tile_pool(name="oacc", bufs=4, space="PSUM") as oaccp,
            tc.tile_pool(name="pt", bufs=pt_bufs) as ptp,
            tc.tile_pool(name="ob", bufs=ob_bufs) as obp,
            tc.tile_pool(name="rd", bufs=rd_bufs) as rdp,
        ):
            if loop_n is not None:
                loop_cm = tc.For_i(
                    0,
                    loop_n,
                    hint_engines=(
                        mybir.EngineType.PE,
                        mybir.EngineType.DVE,
                        mybir.EngineType.Activation,
                        mybir.EngineType.Pool,
                        mybir.EngineType.SP,
                    ),
                )
            else:
                loop_cm = contextlib.nullcontext()
            with loop_cm:
                _emit_body(nc, tc, locals(), frozenset(ablate),
                           pv_delay=pv_delay, group=group, exp_split=exp_split,
                           dma_split=dma_split, qt_gpsimd=qt_gpsimd,
                           vaug_gpsimd=vaug_gpsimd, store_sync=store_sync)
    nc.compile()
    return nc


def _emit_body(nc, tc, env, ablate=frozenset(), pv_delay=1, group=GROUP,
               exp_split=1, dma_split=16, qt_gpsimd=False, vaug_gpsimd=False,
               store_sync=False):
    GROUP = group
    import concourse.mybir as mybir

    fp32 = mybir.dt.float32
    bf16 = mybir.dt.bfloat16
    big, stagep, oaccp, ptp, obp, rdp = (
        env["big"], env["stagep"], env["oaccp"], env["ptp"], env["obp"], env["rdp"]
    )
    qt_d, kt_d, ktv_d, vaug_d, vvaug_d, vs_d, tri_d, mstart_d, smsc_d, o_d = (
        env["qt_d"], env["kt_d"], env["ktv_d"], env["vaug_d"], env["vvaug_d"],
        env["vs_d"], env["tri_d"], env["mstart_d"], env["smsc_d"], env["o_d"],
    )
    visits, first, last, tile_first = (
        env["visits"], env["first"], env["last"], env["tile_first"]
    )
    n_groups = (len(visits) + GROUP - 1) // GROUP
    if True:
        if True:
            qt = big.tile([D, S], bf16)
            kt = big.tile([D, S], bf16)
            ktv = big.tile([D, NVSLOT * BLK], bf16)
            vaug = big.tile([128, NPAIR, D + 1], bf16)
            vvaug = big.tile([128, NVC, D + 1], bf16)
            vs = big.tile([128, 48, 2], fp32)
            tri = big.tile([128, 128], bf16)
            mstart = big.tile([128, 128], bf16)
            smsc = big.tile([128, 1], fp32)

            # small tensors first: group 0's exp/masks/PV depend on them
            nc.sync.dma_start(out=smsc[:], in_=smsc_d[:])
            nc.sync.dma_start(out=tri[:], in_=tri_d[:])
            nc.sync.dma_start(out=mstart[:], in_=mstart_d[:])
            nc.sync.dma_start(out=vs[:], in_=vs_d[:])
            nc.sync.dma_start(out=ktv[:], in_=ktv_d[:])
            nc.sync.dma_start(out=vvaug[:], in_=vvaug_d[:])
            # big loads split fine, in first-use order (kt chunk c at local c,
            # qt pair i from chunk max(0,i-8), vaug chunk c at PV time)
            qt_eng = nc.gpsimd if qt_gpsimd else nc.sync
            vaug_eng = nc.gpsimd if vaug_gpsimd else nc.sync
            for a in range(dma_split):
                sl = slice(a * (S // dma_split), (a + 1) * (S // dma_split))
                nc.sync.dma_start(out=kt[:, sl], in_=kt_d[:, sl])
                qt_eng.dma_start(out=qt[:, sl], in_=qt_d[:, sl])
                sl4 = slice(a * (NPAIR // dma_split), (a + 1) * (NPAIR // dma_split))
                vaug_eng.dma_start(out=vaug[:, sl4], in_=vaug_d[:, sl4])

            oacc_tiles = {}  # pair-group (i//3) -> psum tile [128, 3, 129]
            v_idx = 0  # running vertical-visit index (matches host vs layout)
            pending_pv = []  # software pipeline: PV of group gi-d emitted
            # after S^T of group gi so PE streams while ACT/DVE process gi-d

            for gi in range(n_groups):
                gvis = visits[gi * GROUP : (gi + 1) * GROUP]
                n = len(gvis)
                stage = stagep.tile([128, GROUP * 128], fp32, tag="stage")
                ptt = ptp.tile([128, GROUP * 128], bf16, tag="pt")

                # --- S^T matmuls, batched over runs of consecutive pairs
                # sharing one k-chunk, split at 4-slot (one PSUM bank) bounds.
                # start=True only on the first run per bank (bank zero-region).
                s = 0
                seen_banks = set()
                while s < n:
                    kind, idx, i0 = gvis[s]
                    e = s + 1
                    while (
                        e < n
                        and e % 4 != 0
                        and gvis[e][0] == kind
                        and gvis[e][1] == idx
                        and gvis[e][2] == gvis[e - 1][2] + 1
                    ):
                        e += 1
                    ln = e - s
                    lhsT = (
                        kt[:, idx * 128 : (idx + 1) * 128]
                        if kind == "local"
                        else ktv[:, idx * 128 : (idx + 1) * 128]
                    )
                    bank = s // 4
                    nc.tensor.matmul(
                        stage[:, s * 128 : e * 128],
                        lhsT,
                        qt[:, i0 * 128 : (i0 + ln) * 128],
                        start=bank not in seen_banks,
                        stop=True,
                        skip_group_check=True,
                    )
                    if "dup_st" in ablate:
                        nc.tensor.matmul(
                            stage[:, s * 128 : e * 128],
                            lhsT,
                            qt[:, i0 * 128 : (i0 + ln) * 128],
                            start=False,
                            stop=True,
                            skip_group_check=True,
                        )
                    seen_banks.add(bank)
                    s = e

                if len(pending_pv) >= pv_delay:
                    pending_pv.pop(0)()

                # --- exp for the group (optionally split for finer pipelining)
                if "exp" not in ablate:
                    per = (n + exp_split - 1) // exp_split
                    for es in range(0, n, per):
                        ee = min(n, es + per)
                        for _rep in range(2 if "dup_exp" in ablate else 1):
                            nc.scalar.activation(
                                out=ptt[:, es * 128 : ee * 128],
                                in_=stage[:, es * 128 : ee * 128],
                                func=mybir.ActivationFunctionType.Exp,
                                scale=smsc[:, 0:1],
                            )

                # --- masks
                for s, (kind, idx, i) in enumerate(gvis):
                    if "masks" in ablate:
                        if kind == "vert":
                            v_idx += 1
                        continue
                    sl = slice(s * 128, (s + 1) * 128)
                    if kind == "local" and idx == i:
                        nc.vector.tensor_mul(ptt[:, sl], ptt[:, sl], tri[:])
                    elif kind == "local" and idx == i - 8:
                        nc.vector.tensor_mul(ptt[:, sl], ptt[:, sl], mstart[:])
                    elif kind == "vert":
                        for hh in range(2):
                            hsl = slice(s * 128 + hh * 64, s * 128 + (hh + 1) * 64)
                            nc.vector.tensor_scalar_mul(
                                ptt[:, hsl], ptt[:, hsl], vs[:, v_idx, hh : hh + 1]
                            )
                        v_idx += 1

                # --- PV matmuls + epilogue (deferred one group)
                def make_pv(gi, gvis, ptt):
                    def emit_pv():
                        if "pv" in ablate:
                            return
                        for s, (kind, idx, i) in enumerate(gvis):
                            g = gi * GROUP + s
                            pg = i // 3
                            if pg not in oacc_tiles:
                                oacc_tiles[pg] = oaccp.tile(
                                    [128, 3, D + 1], fp32, tag="oacc", name=f"oacc{pg}"
                                )
                            oacc = oacc_tiles[pg]
                            rhs = vaug[:, idx] if kind == "local" else vvaug[:, idx]
                            nc.tensor.matmul(
                                oacc[:, i % 3],
                                ptt[:, s * 128 : (s + 1) * 128],
                                rhs,
                                start=(g == tile_first[i // 3]),
                                stop=(g == last[i]) and "dup_pv" not in ablate,
                                skip_group_check=True,
                            )
                            if "dup_pv" in ablate:
                                nc.tensor.matmul(
                                    oacc[:, i % 3],
                                    ptt[:, s * 128 : (s + 1) * 128],
                                    rhs,
                                    start=False,
                                    stop=(g == last[i]),
                                    skip_group_check=True,
                                )
                            # epilogue once per oacc tile (after its last
                            # pair closes): a single DVE read of the PSUM
                            # bank, so PE's later PV writes to that bank are
                            # never serialized against mid-tile DVE reads.
                            pg_pairs = [p for p in (3 * pg, 3 * pg + 1, 3 * pg + 2)
                                        if p < NPAIR]
                            if (
                                i == pg_pairs[-1]
                                and g == last[i]
                                and "epi" not in ablate
                            ):
                                osb = obp.tile([128, 3, D + 1], fp32, tag="osb")
                                nc.vector.tensor_copy(osb[:], oacc[:])
                                for jj, pp in enumerate(pg_pairs):
                                    rd = rdp.tile([128, 1], fp32, tag="rd")
                                    nc.vector.reciprocal(
                                        rd[:], osb[:, jj, D : D + 1]
                                    )
                                    ob = obp.tile([128, D], fp32, tag="ob")
                                    nc.vector.tensor_scalar_mul(
                                        ob[:], osb[:, jj, 0:D], rd[:]
                                    )
                                    (nc.sync if store_sync else nc.gpsimd).dma_start(
                                        out=o_d[pp * 128 : (pp + 1) * 128, :],
                                        in_=ob[:],
                                    )
                    return emit_pv

                pending_pv.append(make_pv(gi, gvis, ptt))
            for f in pending_pv:
                f()


def _get_program():
    global _PROGRAM
    if _PROGRAM is None:
        _PROGRAM = _build_program()
    return _PROGRAM


def _host_inputs(q, k, v, sm_scale):
    """Per-core input dicts (host-side shard + layout)."""
    q = np.asarray(q, dtype=np.float32)
    k = np.asarray(k, dtype=np.float32)
    v = np.asarray(v, dtype=np.float32)
    smv = float(np.asarray(sm_scale, dtype=np.float32))

    tri = np.zeros((128, 128), dtype=BF16)
    p = np.arange(128)
    tri[p[:, None] <= p[None, :]] = BF16(1.0)
    mstart = np.zeros((128, 128), dtype=BF16)
    mstart[64:, :64] = BF16(1.0)
    smsc = np.full((128, 1), smv, dtype=np.float32)

    vorder = vert_visit_order()
    ins = []
    for h in range(H):
        r = 7 - h
        qh, kh, vh = q[0, h], k[0, h], v[0, h]
        qt = np.ascontiguousarray(qh.T).astype(BF16)
        kt = np.ascontiguousarray(kh.T).astype(BF16)
        vblocks = [8 * j + r for j in range(NVSLOT)]
        kv = np.concatenate([kh[b * BLK : (b + 1) * BLK] for b in vblocks], axis=0)
        ktv = np.ascontiguousarray(kv.T).astype(BF16)
        vaug = np.concatenate(
            [vh, np.ones((S, 1), np.float32)], axis=1
        ).astype(BF16)  # [4096, 129]
        vaug = np.ascontiguousarray(
            vaug.reshape(NPAIR, 128, D + 1).transpose(1, 0, 2)
        )  # [128, 32, 129]
        vv = np.concatenate([vh[b * BLK : (b + 1) * BLK] for b in vblocks], axis=0)
        vvaug = np.concatenate([vv, np.ones((NVSLOT * BLK, 1), np.float32)], axis=1)
        vvaug = np.ascontiguousarray(
            vvaug.astype(BF16).reshape(NVC, 128, D + 1).transpose(1, 0, 2)
        )  # [128, 3, 129]

        vsc = np.zeros((128, 48, 2), dtype=np.float32)
        for vi, (vc, i) in enumerate(vorder):
            for hh in range(2):
                qb = 2 * i + hh
                slot = 2 * vc + (p >= 64).astype(np.int64)  # per-partition slot
                kb = 8 * slot + r
                vsc[:, vi, hh] = (kb <= qb - 16).astype(np.float32)
        ins.append(
            dict(
                qt=qt, kt=kt, ktv=ktv, vaug=vaug, vvaug=vvaug,
                vs=vsc, tri=tri, mstart=mstart, smsc=smsc,
            )
        )
    return ins


def kernel(q, k, v, sm_scale):
    from concourse.bass_utils import run_bass_kernel_spmd

    nc = _get_program()
    ins = _host_inputs(q, k, v, sm_scale)
    res = run_bass_kernel_spmd(nc, ins, core_ids=list(range(H)))
    out = np.stack([res.results[h]["o"] for h in range(H)], axis=0)[None]
    return out.astype(np.float32)



# revision 2
# speedup vs baseline: 1.2884x; 1.2884x over previous
"""Block-sparse attention (local + vertical-strided causal mask) on 8 TRN2 cores.

Sharding: one head per NeuronCore (H=8, n_cores=8).

Per-core device algorithm (head h, residue r = 7-h):
  The 4096x4096 score matrix is processed at 128x128 granularity:
  "pair" i = q block-rows (2i, 2i+1) (128 q tokens), "chunk" = 128 k tokens
  (2 mask blocks of 64). Local window -> chunks c in [i-8, i] of K itself;
  vertical-strided blocks -> host-gathered K_vert (6 blocks of 64, kb = 8j+r),
  processed as 3 chunks shared by all cores.

  S^T orientation: S^T[k,q] = kT_chunk.T @ qT_pair  (PE, bf16; sm_scale
  pre-folded into qT on host)
  window-start / vert-validity masks: rank-2 additive -C matmuls into the
  same PSUM region (PE); exp underflows those entries to exact 0
  P^T = exp(S^T)                                     (ACT, one call per group)
  diag triangle: multiplicative bf16 mask            (DVE)
  oacc[q, 0:129] += P^T_chunk.T @ [V | 1]_chunk      (PE, PSUM-accumulated)
  col 128 of oacc = softmax denominator; copied PSUM->SBUF (DVE) and stored
  unnormalized; the host divides by the denominator column.
"""

import numpy as np
import ml_dtypes

BF16 = ml_dtypes.bfloat16

H = 8
S = 4096
D = 128
BLK = 64
NB = S // BLK        # 64 block rows
NPAIR = NB // 2      # 32 row pairs
NVSLOT = 6           # usable vertical slots (kb = 8j + r <= 47)
NVC = NVSLOT // 2    # 3 vertical chunks
GROUP = 8            # PSUM staging slots per exp group (8 * 128 f32 = 2 banks)

NEGC = 28672.0       # additive mask constant; bf16-exact, exp() underflows to 0


def make_schedule():
    """Global ordered visit list. visit = (kind, idx, pair)
    kind "local": idx = chunk c (k blocks 2c, 2c+1), pairs i in [c, c+8]
    kind "vert":  idx = vc (K_vert slots 2vc, 2vc+1)
    Vert visits for pairs [c0, c0+8) are inserted right before local chunk
    c0 in {8, 16, 24}, after all their opening local chunks."""
    visits = []
    for c in range(NPAIR):
        if c in (8, 16, 24):
            for vc in range(NVC):
                if 8 * vc + 8 <= c:
                    for i in range(c, c + 8):
                        visits.append(("vert", vc, i))
        for i in range(c, min(c + 8, NPAIR - 1) + 1):
            visits.append(("local", c, i))
    return visits


def mask_visit_order():
    """Visits that need an additive rank-2 mask, in schedule order.
    vert: per-(vc, i) validity; local with idx == i-8: window-start mask."""
    out = []
    for kind, idx, i in make_schedule():
        if kind == "vert" or (kind == "local" and idx == i - 8):
            out.append((kind, idx, i))
    return out


NMASK = len(mask_visit_order())

_PROGRAM = None


def _build_program(pv_delay=3, group=GROUP, stage_bufs=2, pt_bufs=None,
                   ob_bufs=3, kt_split=8, qt_split=4, vaug_split=2):
    if pt_bufs is None:
        pt_bufs = pv_delay + 2
    import concourse.bass as bass
    import concourse.mybir as mybir
    import concourse.tile as tile
    from concourse import bacc

    fp32 = mybir.dt.float32
    bf16 = mybir.dt.bfloat16

    nc = bacc.Bacc("TRN2", target_bir_lowering=False, debug=False, num_devices=H)

    qt_d = nc.dram_tensor("qt", [D, S], bf16, kind="ExternalInput").ap()
    kt_d = nc.dram_tensor("kt", [D, S], bf16, kind="ExternalInput").ap()
    # kaux = [ktv (384 cols) | vvaug (3*129 cols)]
    kaux_d = nc.dram_tensor("kaux", [128, NVSLOT * BLK + NVC * (D + 1)], bf16,
                            kind="ExternalInput").ap()
    vaug_d = nc.dram_tensor("vaug", [128, NPAIR, D + 1], bf16, kind="ExternalInput").ap()
    # vmu = per-mask-visit [2, 128] invalid-indicator lhsT slices + wpat last
    vmu_d = nc.dram_tensor("vmu", [2, (NMASK + 1) * 128], bf16,
                           kind="ExternalInput").ap()
    tri_d = nc.dram_tensor("tri", [128, 128], bf16, kind="ExternalInput").ap()
    o_d = nc.dram_tensor("o", [128, NPAIR, D + 1], fp32, kind="ExternalOutput").ap()

    visits = make_schedule()
    first = {}
    last = {}
    for g, (kind, idx, i) in enumerate(visits):
        first.setdefault(i, g)
        last[i] = g
    # PSUM start_tensor_calc zeroes the full 2KB bank (zero-region), so only
    # the first matmul touching an oacc tile may carry start=True.
    tile_first = {}
    for g, (kind, idx, i) in enumerate(visits):
        tile_first.setdefault(i // 3, g)
    mask_idx = {v: mi for mi, v in enumerate(mask_visit_order())}

    with tile.TileContext(nc) as tc:
        with (
            tc.tile_pool(name="big", bufs=1) as big,
            tc.tile_pool(name="stage", bufs=stage_bufs, space="PSUM") as stagep,
            tc.tile_pool(name="oacc", bufs=4, space="PSUM") as oaccp,
            tc.tile_pool(name="pt", bufs=pt_bufs) as ptp,
            tc.tile_pool(name="ob", bufs=ob_bufs) as obp,
        ):
            _emit_body(nc, tc, locals(), pv_delay=pv_delay, group=group,
                       kt_split=kt_split, qt_split=qt_split, vaug_split=vaug_split)
    nc.compile()
    return nc


def _emit_body(nc, tc, env, pv_delay=3, group=GROUP, kt_split=8, qt_split=4,
               vaug_split=2):
    GROUP = group
    import concourse.mybir as mybir

    fp32 = mybir.dt.float32
    bf16 = mybir.dt.bfloat16
    big, stagep, oaccp, ptp, obp = (
        env["big"], env["stagep"], env["oaccp"], env["ptp"], env["obp"]
    )
    qt_d, kt_d, kaux_d, vaug_d, vmu_d, tri_d, o_d = (
        env["qt_d"], env["kt_d"], env["kaux_d"], env["vaug_d"], env["vmu_d"],
        env["tri_d"], env["o_d"],
    )
    visits, first, last, tile_first, mask_idx = (
        env["visits"], env["first"], env["last"], env["tile_first"],
        env["mask_idx"],
    )
    n_groups = (len(visits) + GROUP - 1) // GROUP

    qt = big.tile([D, S], bf16)
    kt = big.tile([D, S], bf16)
    kaux = big.tile([128, NVSLOT * BLK + NVC * (D + 1)], bf16)
    vaug = big.tile([128, NPAIR, D + 1], bf16)
    vmu = big.tile([2, (NMASK + 1) * 128], bf16)
    tri = big.tile([128, 128], bf16)

    ktv = kaux[:, 0:NVSLOT * BLK]
    wpat = vmu[:, NMASK * 128:(NMASK + 1) * 128]

    # SP queue: tri + first kt chunks first (gate the first S^T), aux next,
    # remaining kt/vaug chunks trail in first-use order.
    ksz = S // kt_split
    nc.sync.dma_start(out=tri[:], in_=tri_d[:])
    for a in range(2):
        sl = slice(a * ksz, (a + 1) * ksz)
        nc.sync.dma_start(out=kt[:, sl], in_=kt_d[:, sl])
    nc.sync.dma_start(out=kaux[:], in_=kaux_d[:])
    nc.sync.dma_start(out=vmu[:], in_=vmu_d[:])
    vsz = NPAIR // vaug_split
    nc.sync.dma_start(out=vaug[:, 0:vsz], in_=vaug_d[:, 0:vsz])
    for a in range(2, kt_split):
        sl = slice(a * ksz, (a + 1) * ksz)
        nc.sync.dma_start(out=kt[:, sl], in_=kt_d[:, sl])
    for b in range(1, vaug_split):
        sl = slice(b * vsz, (b + 1) * vsz)
        nc.sync.dma_start(out=vaug[:, sl], in_=vaug_d[:, sl])
    # ACT queue: qt chunks (issued before any exp reaches the ACT queue)
    qsz = S // qt_split
    for a in range(qt_split):
        sl = slice(a * qsz, (a + 1) * qsz)
        nc.scalar.dma_start(out=qt[:, sl], in_=qt_d[:, sl])

    oacc_tiles = {}  # pair-group (i//3) -> psum tile [128, 3, 129]
    pending_pv = []  # software pipeline: PV of group gi-d emitted
    # after S^T of group gi so PE streams while ACT/DVE process gi-d

    for gi in range(n_groups):
        gvis = visits[gi * GROUP : (gi + 1) * GROUP]
        n = len(gvis)
        stage = stagep.tile([128, GROUP * 128], fp32, tag="stage")
        ptt = ptp.tile([128, GROUP * 128], bf16, tag="pt")

        # --- S^T matmuls, batched over runs of consecutive pairs
        # sharing one k-chunk, split at 4-slot (one PSUM bank) bounds.
        # start=True only on the first run per bank (bank zero-region).
        s = 0
        seen_banks = set()
        while s < n:
            kind, idx, i0 = gvis[s]
            e = s + 1
            while (
                e < n
                and e % 4 != 0
                and gvis[e][0] == kind
                and gvis[e][1] == idx
                and gvis[e][2] == gvis[e - 1][2] + 1
            ):
                e += 1
            ln = e - s
            lhsT = (
                kt[:, idx * 128 : (idx + 1) * 128]
                if kind == "local"
                else ktv[:, idx * 128 : (idx + 1) * 128]
            )
            bank = s // 4
            nc.tensor.matmul(
                stage[:, s * 128 : e * 128],
                lhsT,
                qt[:, i0 * 128 : (i0 + ln) * 128],
                start=bank not in seen_banks,
                stop=True,
                skip_group_check=True,
            )
            seen_banks.add(bank)
            # rank-2 additive masks (window-start / vert validity) for the
            # slots of this run, accumulated into the same PSUM region
            for s2 in range(s, e):
                mv = mask_idx.get(gvis[s2])
                if mv is not None:
                    nc.tensor.matmul(
                        stage[:, s2 * 128 : (s2 + 1) * 128],
                        vmu[:, mv * 128 : (mv + 1) * 128],
                        wpat,
                        start=False,
                        stop=True,
                        skip_group_check=True,
                    )
            s = e

        if len(pending_pv) >= pv_delay:
            pending_pv.pop(0)()

        # --- exp for the group
        nc.scalar.activation(
            out=ptt[:, 0 : n * 128],
            in_=stage[:, 0 : n * 128],
            func=mybir.ActivationFunctionType.Exp,
        )

        # --- diag triangle mask (DVE, multiplicative bf16)
        for s, (kind, idx, i) in enumerate(gvis):
            if kind == "local" and idx == i:
                sl = slice(s * 128, (s + 1) * 128)
                nc.vector.tensor_mul(ptt[:, sl], ptt[:, sl], tri[:])

        # --- PV matmuls + epilogue (deferred pv_delay groups)
        def make_pv(gi, gvis, ptt):
            def emit_pv():
                for s, (kind, idx, i) in enumerate(gvis):
                    g = gi * GROUP + s
                    pg = i // 3
                    if pg not in oacc_tiles:
                        oacc_tiles[pg] = oaccp.tile(
                            [128, 3, D + 1], fp32, tag="oacc", name=f"oacc{pg}"
                        )
                    oacc = oacc_tiles[pg]
                    if kind == "local":
                        rhs = vaug[:, idx]
                    else:
                        off = NVSLOT * BLK + idx * (D + 1)
                        rhs = kaux[:, off : off + (D + 1)]
                    nc.tensor.matmul(
                        oacc[:, i % 3],
                        ptt[:, s * 128 : (s + 1) * 128],
                        rhs,
                        start=(g == tile_first[i // 3]),
                        stop=(g == last[i]),
                        skip_group_check=True,
                    )
                    # epilogue once per oacc tile (after its last pair
                    # closes): one DVE read of the PSUM bank into SBUF,
                    # then an unnormalized store (host divides by col 128).
                    pg_pairs = [p for p in (3 * pg, 3 * pg + 1, 3 * pg + 2)
                                if p < NPAIR]
                    if i == pg_pairs[-1] and g == last[i]:
                        npp = len(pg_pairs)
                        osb = obp.tile([128, 3, D + 1], fp32, tag="osb")
                        nc.vector.tensor_copy(osb[:, 0:npp], oacc[:, 0:npp])
                        nc.sync.dma_start(
                            out=o_d[:, 3 * pg : 3 * pg + npp, :],
                            in_=osb[:, 0:npp],
                        )
            return emit_pv

        pending_pv.append(make_pv(gi, gvis, ptt))
    for f in pending_pv:
        f()


def _get_program():
    global _PROGRAM
    if _PROGRAM is None:
        _PROGRAM = _build_program()
    return _PROGRAM


def _host_inputs(q, k, v, sm_scale):
    """Per-core input dicts (host-side shard + layout)."""
    q = np.asarray(q, dtype=np.float32)
    k = np.asarray(k, dtype=np.float32)
    v = np.asarray(v, dtype=np.float32)
    smv = float(np.asarray(sm_scale, dtype=np.float32))

    p = np.arange(128)
    tri = np.zeros((128, 128), dtype=BF16)
    tri[p[:, None] <= p[None, :]] = BF16(1.0)

    morder = mask_visit_order()
    ins = []
    for h in range(H):
        r = 7 - h
        qh, kh, vh = q[0, h], k[0, h], v[0, h]
        qt = np.ascontiguousarray((qh * smv).T).astype(BF16)
        kt = np.ascontiguousarray(kh.T).astype(BF16)
        vblocks = [8 * j + r for j in range(NVSLOT)]
        kv = np.concatenate([kh[b * BLK : (b + 1) * BLK] for b in vblocks], axis=0)
        ktv = np.ascontiguousarray(kv.T).astype(BF16)  # [128, 384]
        vaug = np.concatenate(
            [vh, np.ones((S, 1), np.float32)], axis=1
        ).astype(BF16)  # [4096, 129]
        vaug = np.ascontiguousarray(
            vaug.reshape(NPAIR, 128, D + 1).transpose(1, 0, 2)
        )  # [128, 32, 129]
        vv = np.concatenate([vh[b * BLK : (b + 1) * BLK] for b in vblocks], axis=0)
        vvaug = np.concatenate([vv, np.ones((NVSLOT * BLK, 1), np.float32)], axis=1)
        vvaug = np.ascontiguousarray(
            vvaug.astype(BF16).reshape(NVC, 128, D + 1).transpose(1, 0, 2)
        )  # [128, 3, 129]
        kaux = np.concatenate(
            [ktv, vvaug.reshape(128, NVC * (D + 1))], axis=1
        )  # [128, 771]

        # vmu: per-mask-visit [2, 128] invalid indicators (u0 for cols<64,
        # u1 for cols>=64), wpat appended last: wj = -C on its col-half.
        vmu = np.zeros((2, (NMASK + 1) * 128), dtype=BF16)
        for mi, (kind, idx, i) in enumerate(morder):
            sl = slice(mi * 128, (mi + 1) * 128)
            if kind == "vert":
                slot = 2 * idx + (p >= 64).astype(np.int64)
                kb = 8 * slot + r
                u0 = (kb > 2 * i - 16).astype(np.float32)       # invalid for qb=2i
                u1 = (kb > 2 * i + 1 - 16).astype(np.float32)   # invalid for qb=2i+1
            else:  # window-start: valid iff (p >= 64 and col < 64)
                u0 = (p < 64).astype(np.float32)
                u1 = np.ones(128, np.float32)
            vmu[0, sl] = u0.astype(BF16)
            vmu[1, sl] = u1.astype(BF16)
        wsl = slice(NMASK * 128, (NMASK + 1) * 128)
        wp = np.zeros((2, 128), np.float32)
        wp[0, :64] = -NEGC
        wp[1, 64:] = -NEGC
        vmu[:, wsl] = wp.astype(BF16)

        ins.append(dict(qt=qt, kt=kt, kaux=kaux, vaug=vaug, vmu=vmu, tri=tri))
    return ins


def kernel(q, k, v, sm_scale):
    from concourse.bass_utils import run_bass_kernel_spmd

    nc = _get_program()
    ins = _host_inputs(q, k, v, sm_scale)
    res = run_bass_kernel_spmd(nc, ins, core_ids=list(range(H)))
    outs = []
    for h in range(H):
        o = res.results[h]["o"]  # [128, NPAIR, 129]
        o = o.transpose(1, 0, 2).reshape(S, D + 1)
        outs.append(o[:, :D] / o[:, D : D + 1])
    out = np.stack(outs, axis=0)[None]
    return out.astype(np.float32)


# revision 4
# speedup vs baseline: 1.3441x; 1.0433x over previous
"""Block-sparse attention (local + vertical-strided causal mask) on 8 TRN2 cores.

Sharding: one head per NeuronCore (H=8, n_cores=8).

Per-core device algorithm (head h, residue r = 7-h):
  The 4096x4096 score matrix is processed at 128x128 granularity:
  "pair" i = q block-rows (2i, 2i+1) (128 q tokens), "chunk" = 128 k tokens
  (2 mask blocks of 64). Local window -> chunks c in [i-8, i] of K itself;
  vertical-strided blocks -> host-gathered K_vert (6 blocks of 64, kb = 8j+r),
  processed as 3 chunks shared by all cores.

  S^T orientation: S^T[k,q] = kT_chunk.T @ qT_pair  (PE, bf16; sm_scale
  pre-folded into qT on host)
  window-start / vert-validity masks: rank-2 additive -C matmuls into the
  same PSUM region (PE); exp underflows those entries to exact 0
  P^T = exp(S^T)                                     (ACT, one call per group)
  diag triangle: multiplicative bf16 mask            (DVE)
  oacc[q, 0:129] += P^T_chunk.T @ [V | 1]_chunk      (PE, PSUM-accumulated)
  col 128 of oacc = softmax denominator; copied PSUM->SBUF (DVE) and stored
  unnormalized; the host divides by the denominator column.
"""

import numpy as np
import ml_dtypes

BF16 = ml_dtypes.bfloat16

H = 8
S = 4096
D = 128
BLK = 64
NB = S // BLK        # 64 block rows
NPAIR = NB // 2      # 32 row pairs
NVSLOT = 6           # usable vertical slots (kb = 8j + r <= 47)
NVC = NVSLOT // 2    # 3 vertical chunks
GROUP = 8            # PSUM staging slots per exp group (8 * 128 f32 = 2 banks)

NEGC = 28672.0       # additive mask constant; bf16-exact, exp() underflows to 0


def make_schedule():
    """Global ordered visit list. visit = (kind, idx, pair)
    kind "local": idx = chunk c (k blocks 2c, 2c+1), pairs i in [c, c+8]
    kind "vert":  idx = vc (K_vert slots 2vc, 2vc+1)
    Vert visits for pairs [c0, c0+8) are inserted right before local chunk
    c0 in {8, 16, 24}, after all their opening local chunks."""
    visits = []
    for c in range(NPAIR):
        if c in (8, 16, 24):
            for vc in range(NVC):
                if 8 * vc + 8 <= c:
                    for i in range(c, c + 8):
                        visits.append(("vert", vc, i))
        for i in range(c, min(c + 8, NPAIR - 1) + 1):
            visits.append(("local", c, i))
    return visits


def mask_visit_order():
    """Visits that need an additive rank-2 mask, in schedule order.
    vert: per-(vc, i) validity — pruned when every head's slots are valid
    (kb_max = 16*vc+8+r <= 2i-16 for all r < 8, i.e. i >= 8*vc+16);
    local with idx == i-8: window-start mask (always needed)."""
    out = []
    for kind, idx, i in make_schedule():
        if kind == "vert" and i < 8 * idx + 16:
            out.append((kind, idx, i))
        elif kind == "local" and idx == i - 8:
            out.append((kind, idx, i))
    return out


NMASK = len(mask_visit_order())

_PROGRAM = None


def _build_program(pv_delay=3, group=GROUP, stage_bufs=2, pt_bufs=None,
                   ob_bufs=3, kt_split=8, qt_split=4, vaug_split=2):
    if pt_bufs is None:
        pt_bufs = pv_delay + 2
    import concourse.bass as bass
    import concourse.mybir as mybir
    import concourse.tile as tile
    from concourse import bacc

    fp32 = mybir.dt.float32
    bf16 = mybir.dt.bfloat16

    nc = bacc.Bacc("TRN2", target_bir_lowering=False, debug=False, num_devices=H)

    qt_d = nc.dram_tensor("qt", [D, S], bf16, kind="ExternalInput").ap()
    kt_d = nc.dram_tensor("kt", [D, S], bf16, kind="ExternalInput").ap()
    # kaux = [ktv (384 cols) | vvaug (3*129 cols)]
    kaux_d = nc.dram_tensor("kaux", [128, NVSLOT * BLK + NVC * (D + 1)], bf16,
                            kind="ExternalInput").ap()
    vaug_d = nc.dram_tensor("vaug", [128, NPAIR, D + 1], bf16, kind="ExternalInput").ap()
    # vmu = per-mask-visit [2, 128] invalid-indicator lhsT slices + wpat last
    vmu_d = nc.dram_tensor("vmu", [2, (NMASK + 1) * 128], bf16,
                           kind="ExternalInput").ap()
    tri_d = nc.dram_tensor("tri", [128, 128], bf16, kind="ExternalInput").ap()
    o_d = nc.dram_tensor("o", [128, NPAIR, D + 1], fp32, kind="ExternalOutput").ap()

    visits = make_schedule()
    first = {}
    last = {}
    for g, (kind, idx, i) in enumerate(visits):
        first.setdefault(i, g)
        last[i] = g
    # PSUM start_tensor_calc zeroes the full 2KB bank (zero-region), so only
    # the first matmul touching an oacc tile may carry start=True.
    tile_first = {}
    for g, (kind, idx, i) in enumerate(visits):
        tile_first.setdefault(i // 3, g)
    mask_idx = {v: mi for mi, v in enumerate(mask_visit_order())}

    with tile.TileContext(nc) as tc:
        with (
            tc.tile_pool(name="big", bufs=1) as big,
            tc.tile_pool(name="stage", bufs=stage_bufs, space="PSUM") as stagep,
            tc.tile_pool(name="oacc", bufs=4, space="PSUM") as oaccp,
            tc.tile_pool(name="pt", bufs=pt_bufs) as ptp,
            tc.tile_pool(name="ob", bufs=ob_bufs) as obp,
        ):
            _emit_body(nc, tc, locals(), pv_delay=pv_delay, group=group,
                       kt_split=kt_split, qt_split=qt_split, vaug_split=vaug_split)
    nc.compile()
    return nc


def _emit_body(nc, tc, env, pv_delay=3, group=GROUP, kt_split=8, qt_split=4,
               vaug_split=2):
    GROUP = group
    import concourse.mybir as mybir

    fp32 = mybir.dt.float32
    bf16 = mybir.dt.bfloat16
    big, stagep, oaccp, ptp, obp = (
        env["big"], env["stagep"], env["oaccp"], env["ptp"], env["obp"]
    )
    qt_d, kt_d, kaux_d, vaug_d, vmu_d, tri_d, o_d = (
        env["qt_d"], env["kt_d"], env["kaux_d"], env["vaug_d"], env["vmu_d"],
        env["tri_d"], env["o_d"],
    )
    visits, first, last, tile_first, mask_idx = (
        env["visits"], env["first"], env["last"], env["tile_first"],
        env["mask_idx"],
    )
    n_groups = (len(visits) + GROUP - 1) // GROUP

    qt = big.tile([D, S], bf16)
    kt = big.tile([D, S], bf16)
    kaux = big.tile([128, NVSLOT * BLK + NVC * (D + 1)], bf16)
    vaug = big.tile([128, NPAIR, D + 1], bf16)
    vmu = big.tile([2, (NMASK + 1) * 128], bf16)
    tri = big.tile([128, 128], bf16)

    ktv = kaux[:, 0:NVSLOT * BLK]
    wpat = vmu[:, NMASK * 128:(NMASK + 1) * 128]

    # SP queue: small first chunks gate the first S^T; the rest trails in
    # first-use order. ACT queue carries qt (issued before any exp).
    def kt_dma(a, b):
        nc.sync.dma_start(out=kt[:, a:b], in_=kt_d[:, a:b])

    def vaug_dma(a, b):
        nc.sync.dma_start(out=vaug[:, a:b], in_=vaug_d[:, a:b])

    kt_dma(0, 128)
    kt_dma(128, 512)
    nc.sync.dma_start(out=vmu[:], in_=vmu_d[:])
    vaug_dma(0, 8)
    nc.sync.dma_start(out=tri[:], in_=tri_d[:])
    kt_dma(512, 1024)
    vaug_dma(8, 16)
    kt_dma(1024, 2048)
    nc.sync.dma_start(out=kaux[:], in_=kaux_d[:])
    kt_dma(2048, 3072)
    vaug_dma(16, 24)
    kt_dma(3072, 4096)
    vaug_dma(24, 32)
    for a, b in ((0, 512), (512, 1024), (1024, 2048), (2048, 3072), (3072, 4096)):
        nc.scalar.dma_start(out=qt[:, a:b], in_=qt_d[:, a:b])

    # PE p-state warmup: stream throwaway matmuls on the first kt chunk so
    # the array is ramping while the remaining inputs arrive.
    warm = stagep.tile([128, GROUP * 128], fp32, tag="stage")
    for w in range(12):
        nc.tensor.matmul(
            warm[:, (w % 8) * 128 : (w % 8 + 1) * 128],
            kt[:, 0:128],
            kt[:, 0:128],
            start=True,
            stop=True,
            skip_group_check=True,
        )

    oacc_tiles = {}  # pair-group (i//3) -> psum tile [128, 3, 129]
    pending_pv = []  # software pipeline: PV of group gi-d emitted
    # after S^T of group gi so PE streams while ACT/DVE process gi-d

    for gi in range(n_groups):
        gvis = visits[gi * GROUP : (gi + 1) * GROUP]
        n = len(gvis)
        stage = stagep.tile([128, GROUP * 128], fp32, tag="stage")
        ptt = ptp.tile([128, GROUP * 128], bf16, tag="pt")

        # --- S^T matmuls, batched over runs of consecutive pairs
        # sharing one k-chunk, split at 4-slot (one PSUM bank) bounds.
        # start=True only on the first run per bank (bank zero-region).
        s = 0
        seen_banks = set()
        while s < n:
            kind, idx, i0 = gvis[s]
            e = s + 1
            while (
                e < n
                and e % 4 != 0
                and gvis[e][0] == kind
                and gvis[e][1] == idx
                and gvis[e][2] == gvis[e - 1][2] + 1
            ):
                e += 1
            ln = e - s
            lhsT = (
                kt[:, idx * 128 : (idx + 1) * 128]
                if kind == "local"
                else ktv[:, idx * 128 : (idx + 1) * 128]
            )
            bank = s // 4
            nc.tensor.matmul(
                stage[:, s * 128 : e * 128],
                lhsT,
                qt[:, i0 * 128 : (i0 + ln) * 128],
                start=bank not in seen_banks,
                stop=True,
                skip_group_check=True,
            )
            seen_banks.add(bank)
            # rank-2 additive masks (window-start / vert validity) for the
            # slots of this run, accumulated into the same PSUM region
            for s2 in range(s, e):
                mv = mask_idx.get(gvis[s2])
                if mv is not None:
                    nc.tensor.matmul(
                        stage[:, s2 * 128 : (s2 + 1) * 128],
                        vmu[:, mv * 128 : (mv + 1) * 128],
                        wpat,
                        start=False,
                        stop=True,
                        skip_group_check=True,
                    )
            s = e

        if len(pending_pv) >= pv_delay:
            pending_pv.pop(0)()

        # --- exp for the group
        nc.scalar.activation(
            out=ptt[:, 0 : n * 128],
            in_=stage[:, 0 : n * 128],
            func=mybir.ActivationFunctionType.Exp,
        )

        # --- diag triangle mask (DVE, multiplicative bf16)
        for s, (kind, idx, i) in enumerate(gvis):
            if kind == "local" and idx == i:
                sl = slice(s * 128, (s + 1) * 128)
                nc.vector.tensor_mul(ptt[:, sl], ptt[:, sl], tri[:])

        # --- PV matmuls + epilogue (deferred pv_delay groups)
        def make_pv(gi, gvis, ptt):
            def emit_pv():
                for s, (kind, idx, i) in enumerate(gvis):
                    g = gi * GROUP + s
                    pg = i // 3
                    if pg not in oacc_tiles:
                        oacc_tiles[pg] = oaccp.tile(
                            [128, 3, D + 1], fp32, tag="oacc", name=f"oacc{pg}"
                        )
                    oacc = oacc_tiles[pg]
                    if kind == "local":
                        rhs = vaug[:, idx]
                    else:
                        off = NVSLOT * BLK + idx * (D + 1)
                        rhs = kaux[:, off : off + (D + 1)]
                    nc.tensor.matmul(
                        oacc[:, i % 3],
                        ptt[:, s * 128 : (s + 1) * 128],
                        rhs,
                        start=(g == tile_first[i // 3]),
                        stop=(g == last[i]),
                        skip_group_check=True,
                    )
                    # epilogue once per oacc tile (after its last pair
                    # closes): one DVE read of the PSUM bank into SBUF,
                    # then an unnormalized store (host divides by col 128).
                    pg_pairs = [p for p in (3 * pg, 3 * pg + 1, 3 * pg + 2)
                                if p < NPAIR]
                    if i == pg_pairs[-1] and g == last[i]:
                        npp = len(pg_pairs)
                        osb = obp.tile([128, 3, D + 1], fp32, tag="osb")
                        nc.vector.tensor_copy(osb[:, 0:npp], oacc[:, 0:npp])
                        nc.sync.dma_start(
                            out=o_d[:, 3 * pg : 3 * pg + npp, :],
                            in_=osb[:, 0:npp],
                        )
            return emit_pv

        pending_pv.append(make_pv(gi, gvis, ptt))
    for f in pending_pv:
        f()


def _get_program():
    global _PROGRAM
    if _PROGRAM is None:
        _PROGRAM = _build_program()
    return _PROGRAM


def _host_inputs(q, k, v, sm_scale):
    """Per-core input dicts (host-side shard + layout)."""
    q = np.asarray(q, dtype=np.float32)
    k = np.asarray(k, dtype=np.float32)
    v = np.asarray(v, dtype=np.float32)
    smv = float(np.asarray(sm_scale, dtype=np.float32))

    p = np.arange(128)
    tri = np.zeros((128, 128), dtype=BF16)
    tri[p[:, None] <= p[None, :]] = BF16(1.0)

    morder = mask_visit_order()
    ins = []
    for h in range(H):
        r = 7 - h
        qh, kh, vh = q[0, h], k[0, h], v[0, h]
        qt = np.ascontiguousarray((qh * smv).T).astype(BF16)
        kt = np.ascontiguousarray(kh.T).astype(BF16)
        vblocks = [8 * j + r for j in range(NVSLOT)]
        kv = np.concatenate([kh[b * BLK : (b + 1) * BLK] for b in vblocks], axis=0)
        ktv = np.ascontiguousarray(kv.T).astype(BF16)  # [128, 384]
        vaug = np.concatenate(
            [vh, np.ones((S, 1), np.float32)], axis=1
        ).astype(BF16)  # [4096, 129]
        vaug = np.ascontiguousarray(
            vaug.reshape(NPAIR, 128, D + 1).transpose(1, 0, 2)
        )  # [128, 32, 129]
        vv = np.concatenate([vh[b * BLK : (b + 1) * BLK] for b in vblocks], axis=0)
        vvaug = np.concatenate([vv, np.ones((NVSLOT * BLK, 1), np.float32)], axis=1)
        vvaug = np.ascontiguousarray(
            vvaug.astype(BF16).reshape(NVC, 128, D + 1).transpose(1, 0, 2)
        )  # [128, 3, 129]
        kaux = np.concatenate(
            [ktv, vvaug.reshape(128, NVC * (D + 1))], axis=1
        )  # [128, 771]

        # vmu: per-mask-visit [2, 128] invalid indicators (u0 for cols<64,
        # u1 for cols>=64), wpat appended last: wj = -C on its col-half.
        vmu = np.zeros((2, (NMASK + 1) * 128), dtype=BF16)
        for mi, (kind, idx, i) in enumerate(morder):
            sl = slice(mi * 128, (mi + 1) * 128)
            if kind == "vert":
                slot = 2 * idx + (p >= 64).astype(np.int64)
                kb = 8 * slot + r
                u0 = (kb > 2 * i - 16).astype(np.float32)       # invalid for qb=2i
                u1 = (kb > 2 * i + 1 - 16).astype(np.float32)   # invalid for qb=2i+1
            else:  # window-start: valid iff (p >= 64 and col < 64)
                u0 = (p < 64).astype(np.float32)
                u1 = np.ones(128, np.float32)
            vmu[0, sl] = u0.astype(BF16)
            vmu[1, sl] = u1.astype(BF16)
        wsl = slice(NMASK * 128, (NMASK + 1) * 128)
        wp = np.zeros((2, 128), np.float32)
        wp[0, :64] = -NEGC
        wp[1, 64:] = -NEGC
        vmu[:, wsl] = wp.astype(BF16)

        ins.append(dict(qt=qt, kt=kt, kaux=kaux, vaug=vaug, vmu=vmu, tri=tri))
    return ins


def kernel(q, k, v, sm_scale):
    from concourse.bass_utils import run_bass_kernel_spmd

    nc = _get_program()
    ins = _host_inputs(q, k, v, sm_scale)
    res = run_bass_kernel_spmd(nc, ins, core_ids=list(range(H)))
    outs = []
    for h in range(H):
        o = res.results[h]["o"]  # [128, NPAIR, 129]
        o = o.transpose(1, 0, 2).reshape(S, D + 1)
        outs.append(o[:, :D] / o[:, D : D + 1])
    out = np.stack(outs, axis=0)[None]
    return out.astype(np.float32)
